# revision 7
# baseline (speedup 1.0000x reference)
"""Causal self-attention (B=4, T=2048, C=1024, H=16) on 8 Trainium2 NeuronCores.

Sharding: core = (b, g) with b = core//2 (batch), g = core%2 (head group of 8
heads / 512 features).  Each core computes its batch's attention for its 8
heads plus the partial output projection for its feature slice; the host sums
the two partials per batch and adds the projection bias.

Per-core kernel (all shapes hardcoded):
  inputs  xT (1024, 2048) = x[b].T          fp32r
          wqT/wkT/wvT (1024, 512) = W[g].T  fp32r
          wpT (512, 1024) = Wp[:, g].T      fp32r
          bqh/bkh (128, 4), bvh (1, 512)    fp32
  output  out (2048, 1024) partial          fp32

Stages: QT/KT = W.T@xT (transposed layout, per-partition bias), V natural
layout with a fused ones column; per head ST = K Q^T (K=64 contraction), exp
on ScalarE with 1/sqrt(D) folded into the activation scale, causal handled by
skipping blocks above the diagonal + a triangular mask on diagonal blocks;
AV as YT' = V_aug.T @ E accumulated in PSUM (row 64 = softmax denominator),
normalized via reciprocal + partition_broadcast; projection from YT layout.
"""

import sys

if "/opt/trn_rl_repo" not in sys.path:
    sys.path.insert(0, "/opt/trn_rl_repo")

import numpy as np

B, T, C, H = 4, 2048, 1024, 16
D = C // H          # 64 head dim
GH = H // 2         # 8 heads per core
CG = C // 2         # 512 features per head group
P = 128             # partitions
NBLK = 512          # free-dim block (t-block / i-block)
N_CORES = 8

_CACHE = {}
RUN_KWARGS = {}     # test harness can set {"trace": True, ...}
LAST_RESULT = [None]


def _build_nc(t=T):
    import concourse.mybir as mybir
    from concourse import bacc
    from concourse.tile import TileContext

    f32 = mybir.dt.float32
    f32r = mybir.dt.float32r
    bf16 = mybir.dt.bfloat16

    nt = t // P            # t-tiles
    nib = t // NBLK        # i-blocks
    ck = C // P            # 8 contraction tiles over C
    nm = CG // P           # 4 c'-tiles per group
    blk_t = NBLK // P      # 4 t-tiles per block

    nc = bacc.Bacc("TRN2", target_bir_lowering=False, num_devices=N_CORES)

    xT = nc.dram_tensor("xT", (C, t), f32r, kind="ExternalInput")
    wqT = nc.dram_tensor("wqT", (C, CG), f32r, kind="ExternalInput")
    wkT = nc.dram_tensor("wkT", (C, CG), f32r, kind="ExternalInput")
    wvT = nc.dram_tensor("wvT", (C, CG), f32r, kind="ExternalInput")
    wpT = nc.dram_tensor("wpT", (CG, C), f32r, kind="ExternalInput")
    bqh = nc.dram_tensor("bqh", (P, nm), f32, kind="ExternalInput")
    bkh = nc.dram_tensor("bkh", (P, nm), f32, kind="ExternalInput")
    bvh = nc.dram_tensor("bvh", (1, CG), f32, kind="ExternalInput")
    out = nc.dram_tensor("out", (t, C), f32, kind="ExternalOutput")

    from contextlib import ExitStack

    with TileContext(nc) as tc, ExitStack() as es:
        pp = es.enter_context(tc.tile_pool(name="persist", bufs=1))
        xtpool = es.enter_context(tc.tile_pool(name="xt", bufs=10))
        epool = es.enter_context(tc.tile_pool(name="e", bufs=6))
        ytpool = es.enter_context(tc.tile_pool(name="yt", bufs=8))
        opool = es.enter_context(tc.tile_pool(name="osb", bufs=4))
        npool = es.enter_context(tc.tile_pool(name="nrm", bufs=2))
        if True:
            # ---- constants ----
            bq_sb = pp.tile([P, nm], f32, tag="bq_sb", name="bq_sb")
            nc.sync.dma_start(out=bq_sb, in_=bqh[:, :])
            bk_sb = pp.tile([P, nm], f32, tag="bk_sb", name="bk_sb")
            nc.sync.dma_start(out=bk_sb, in_=bkh[:, :])
            bv_row = pp.tile([1, CG], f32, tag="bv_row", name="bv_row")
            nc.sync.dma_start(out=bv_row, in_=bvh[:, :])
            bv_bc = pp.tile([P, CG], f32, tag="bv_bc", name="bv_bc")
            nc.gpsimd.partition_broadcast(bv_bc, bv_row)
            # tri[p, y] = 1 if y >= p else 0  (keep i_local >= j_local)
            tri = pp.tile([P, P], bf16, tag="tri", name="tri")
            nc.gpsimd.memset(tri, 1.0)
            nc.gpsimd.affine_select(
                out=tri, in_=tri, compare_op=mybir.AluOpType.is_ge,
                fill=0.0, base=0, pattern=[[1, P]], channel_multiplier=-1,
            )

            # ---- weights ----
            wq_sb, wk_sb, wv_sb = [], [], []
            for k in range(ck):
                for lst, wt, nm_ in ((wq_sb, wqT, "wq"), (wk_sb, wkT, "wk"), (wv_sb, wvT, "wv")):
                    tl = pp.tile([P, CG], f32r, tag=f"{nm_}{k}", name=f"{nm_}{k}")
                    nc.sync.dma_start(out=tl, in_=wt[k * P:(k + 1) * P, :])
                    lst.append(tl)
            wp_sb = []
            for p_ in range(nm):
                tl = pp.tile([P, C], f32r, tag=f"wp{p_}", name=f"wp{p_}")
                nc.sync.dma_start(out=tl, in_=wpT[p_ * P:(p_ + 1) * P, :])
                wp_sb.append(tl)

            # ---- stage 1: QKV ----
            qt_sb = [pp.tile([P, t], f32r, tag=f"qt{m}", name=f"qt{m}") for m in range(nm)]
            kt_sb = [pp.tile([P, t], f32r, tag=f"kt{m}", name=f"kt{m}") for m in range(nm)]
            # v tiles: (P, 8*65) bf16 — per head 64 V columns + a ones column
            v_sb = [pp.tile([P, GH * (D + 1)], bf16, tag=f"v{i}", name=f"v{i}") for i in range(nt)]

            with tc.tile_pool(name="qkv_ps", bufs=8, space="PSUM") as qkvpool:
                for nb in range(nib):
                    ts_ = slice(nb * NBLK, (nb + 1) * NBLK)
                    xts = []
                    for k in range(ck):
                        xt_ = xtpool.tile([P, NBLK], f32r, tag="xt", name=f"xt{nb}_{k}")
                        nc.sync.dma_start(out=xt_, in_=xT[k * P:(k + 1) * P, ts_])
                        xts.append(xt_)
                    ps_q = [qkvpool.tile([P, NBLK], f32, tag="ps", name=f"psq{nb}_{m}") for m in range(nm)]
                    ps_k = [qkvpool.tile([P, NBLK], f32, tag="ps", name=f"psk{nb}_{m}") for m in range(nm)]
                    for k in range(ck):
                        for m in range(nm):
                            nc.tensor.matmul(ps_q[m], wq_sb[k][:, m * P:(m + 1) * P], xts[k],
                                             start=(k == 0), stop=(k == ck - 1))
                        for m in range(nm):
                            nc.tensor.matmul(ps_k[m], wk_sb[k][:, m * P:(m + 1) * P], xts[k],
                                             start=(k == 0), stop=(k == ck - 1))
                    for m in range(nm):
                        nc.vector.tensor_scalar_add(qt_sb[m][:, ts_], ps_q[m], bq_sb[:, m:m + 1])
                        nc.vector.tensor_scalar_add(kt_sb[m][:, ts_], ps_k[m], bk_sb[:, m:m + 1])
                    ps_v = [qkvpool.tile([P, NBLK], f32, tag="ps", name=f"psv{nb}_{i}") for i in range(blk_t)]
                    for k in range(ck):
                        for i in range(blk_t):
                            nc.tensor.matmul(ps_v[i], xts[k][:, i * P:(i + 1) * P], wv_sb[k],
                                             start=(k == 0), stop=(k == ck - 1))
                    for i in range(blk_t):
                        vt = v_sb[nb * blk_t + i]
                        v3 = vt.rearrange("p (h e) -> p h e", e=D + 1)
                        nc.vector.tensor_add(
                            v3[:, :, 0:D],
                            ps_v[i].rearrange("p (h d) -> p h d", d=D),
                            bv_bc.rearrange("p (h d) -> p h d", d=D),
                        )
                        nc.vector.memset(v3[:, :, D:D + 1], 1.0)

            # ---- stage 2+3: attention + projection per i-block ----
            stpool = es.enter_context(tc.tile_pool(name="st_ps", bufs=3, space="PSUM"))
            avpool = es.enter_context(tc.tile_pool(name="av_ps", bufs=2, space="PSUM"))
            pjpool = es.enter_context(tc.tile_pool(name="pj_ps", bufs=2, space="PSUM"))
            for ib in range(nib):
                jt_max = blk_t * (ib + 1)
                is_ = slice(ib * NBLK, (ib + 1) * NBLK)
                yt_tiles = []
                for h in range(GH):
                    m, r = h // 2, (h % 2) * D
                    ytps = avpool.tile([D + 1, NBLK], f32, tag="ytps", name=f"ytps{ib}_{h}")
                    e_tiles = []
                    for jt in range(jt_max):
                        st = stpool.tile([P, NBLK], f32, tag="st", name=f"st{ib}_{h}_{jt}")
                        nc.tensor.matmul(
                            st,
                            kt_sb[m][r:r + D, jt * P:(jt + 1) * P],
                            qt_sb[m][r:r + D, is_],
                            start=True, stop=True,
                        )
                        e = epool.tile([P, NBLK], bf16, tag="e", name=f"e{ib}_{h}_{jt}")
                        o = jt * P - ib * NBLK
                        if o < 0:
                            nc.scalar.activation(e, st, mybir.ActivationFunctionType.Exp,
                                                 scale=0.125)
                        else:
                            if o > 0:
                                nc.vector.memset(e[:, 0:o], 0.0)
                            nc.scalar.activation(e[:, o:NBLK], st[:, o:NBLK],
                                                 mybir.ActivationFunctionType.Exp, scale=0.125)
                            nc.vector.tensor_mul(e[:, o:o + P], e[:, o:o + P], tri)
                        e_tiles.append(e)
                        if jt > 0:
                            nc.tensor.matmul(ytps, v_sb[jt - 1][:, h * (D + 1):(h + 1) * (D + 1)],
                                             e_tiles[jt - 1], start=(jt - 1 == 0), stop=False)
                    nc.tensor.matmul(ytps, v_sb[jt_max - 1][:, h * (D + 1):(h + 1) * (D + 1)],
                                     e_tiles[jt_max - 1], start=(jt_max - 1 == 0), stop=True)
                    # normalize: row D of ytps is the softmax denominator
                    recip = npool.tile([1, NBLK], f32, tag="recip", name=f"rc{ib}_{h}")
                    nc.vector.reciprocal(recip, ytps[D:D + 1, :])
                    bc = npool.tile([D, NBLK], f32, tag="bc", name=f"bc{ib}_{h}")
                    nc.gpsimd.partition_broadcast(bc, recip)
                    if h % 2 == 0:
                        yt_cur = ytpool.tile([P, NBLK], f32r, tag="yt", name=f"yt{ib}_{h // 2}")
                        yt_tiles.append(yt_cur)
                    nc.vector.tensor_mul(yt_tiles[h // 2][r:r + D, :], ytps[0:D, :], bc)
                # projection for this i-block's 4 t-tiles
                for i in range(blk_t):
                    tt = ib * blk_t + i
                    for cb in range(C // NBLK):
                        pj = pjpool.tile([P, NBLK], f32, tag="pj", name=f"pj{tt}_{cb}")
                        for p_ in range(nm):
                            nc.tensor.matmul(pj, yt_tiles[p_][:, i * P:(i + 1) * P],
                                             wp_sb[p_][:, cb * NBLK:(cb + 1) * NBLK],
                                             start=(p_ == 0), stop=(p_ == nm - 1))
                        ot = opool.tile([P, NBLK], f32, tag="osb", name=f"ot{tt}_{cb}")
                        nc.vector.tensor_copy(out=ot, in_=pj)
                        nc.sync.dma_start(out=out[tt * P:(tt + 1) * P, cb * NBLK:(cb + 1) * NBLK],
                                          in_=ot)

    nc.compile()
    return nc


def _get_nc(t=T):
    if t not in _CACHE:
        _CACHE[t] = _build_nc(t)
    return _CACHE[t]


def kernel(x, Wq, bq, Wk, bk, Wv, bv, Wp, bp):
    from concourse import bass_utils

    x = np.asarray(x, dtype=np.float32)
    Wq = np.asarray(Wq, dtype=np.float32)
    Wk = np.asarray(Wk, dtype=np.float32)
    Wv = np.asarray(Wv, dtype=np.float32)
    Wp = np.asarray(Wp, dtype=np.float32)
    bq = np.asarray(bq, dtype=np.float32)
    bk = np.asarray(bk, dtype=np.float32)
    bv = np.asarray(bv, dtype=np.float32)
    bp = np.asarray(bp, dtype=np.float32)

    nc = _get_nc()

    in_maps = []
    for core in range(N_CORES):
        b, g = core // 2, core % 2
        gs = slice(g * CG, (g + 1) * CG)
        in_maps.append({
            "xT": np.ascontiguousarray(x[b].T),
            "wqT": np.ascontiguousarray(Wq[gs, :].T),
            "wkT": np.ascontiguousarray(Wk[gs, :].T),
            "wvT": np.ascontiguousarray(Wv[gs, :].T),
            "wpT": np.ascontiguousarray(Wp[:, gs].T),
            "bqh": np.ascontiguousarray(bq[gs].reshape(CG // P, P).T),
            "bkh": np.ascontiguousarray(bk[gs].reshape(CG // P, P).T),
            "bvh": bv[gs].reshape(1, CG),
        })

    res = bass_utils.run_bass_kernel_spmd(nc, in_maps, core_ids=list(range(N_CORES)),
                                          **RUN_KWARGS)
    LAST_RESULT[0] = res
    y = np.empty((B, T, C), dtype=np.float32)
    for b in range(B):
        y[b] = res.results[2 * b]["out"] + res.results[2 * b + 1]["out"] + bp
    return y


# revision 9
# speedup vs baseline: 1.1407x; 1.1407x over previous
"""Causal self-attention (B=4, T=2048, C=1024, H=16) on 8 Trainium2 NeuronCores.

Sharding: core = (b, g) with b = core//2 (batch), g = core%2 (head group of 8
heads / 512 features).  Each core computes its batch's attention for its 8
heads plus the partial output projection for its feature slice; the host sums
the two partials per batch and adds the projection bias.

Per-core kernel (all shapes hardcoded):
  inputs  xT (1024, 2048) = x[b].T          bf16
          wqT/wkT/wvT (1024, 512) = W[g].T  bf16
          wpT (512, 1024) = Wp[:, g].T      bf16
          bqh/bkh (128, 4), bvh (1, 512)    fp32
  output  out (2048, 1024) partial          fp32

Stages: QT/KT = W.T@xT (transposed layout, per-partition bias), V natural
layout followed by a 64-wide ones block; per head ST = K Q^T (K=64
contraction), exp on ScalarE with 1/sqrt(D) folded into the activation
scale, causal handled by skipping blocks above the diagonal + a triangular
mask on diagonal blocks; AV as YT' = [V_h | ones64].T @ E accumulated in
PSUM so rows 64:128 all hold the softmax denominator (reciprocal runs on 64
lanes); projection from the YT layout, host adds the two partials + bias.
"""

import sys

if "/opt/trn_rl_repo" not in sys.path:
    sys.path.insert(0, "/opt/trn_rl_repo")

import numpy as np

B, T, C, H = 4, 2048, 1024, 16
D = C // H          # 64 head dim
GH = H // 2         # 8 heads per core
CG = C // 2         # 512 features per head group
P = 128             # partitions
NBLK = 512          # free-dim block (t-block / i-block)
N_CORES = 8

_CACHE = {}
RUN_KWARGS = {}     # test harness can set {"trace": True, ...}
LAST_RESULT = [None]


def _build_nc(t=T):
    import concourse.mybir as mybir
    from concourse import bacc
    from concourse.tile import TileContext

    f32 = mybir.dt.float32
    bf16 = mybir.dt.bfloat16

    nt = t // P            # t-tiles
    nib = t // NBLK        # i-blocks
    ck = C // P            # 8 contraction tiles over C
    nm = CG // P           # 4 c'-tiles per group
    blk_t = NBLK // P      # 4 t-tiles per block

    nc = bacc.Bacc("TRN2", target_bir_lowering=False, num_devices=N_CORES)

    xT = nc.dram_tensor("xT", (C, t), bf16, kind="ExternalInput")
    wqT = nc.dram_tensor("wqT", (C, CG), bf16, kind="ExternalInput")
    wkT = nc.dram_tensor("wkT", (C, CG), bf16, kind="ExternalInput")
    wvT = nc.dram_tensor("wvT", (C, CG), bf16, kind="ExternalInput")
    wpT = nc.dram_tensor("wpT", (CG, C), bf16, kind="ExternalInput")
    bqh = nc.dram_tensor("bqh", (P, nm), f32, kind="ExternalInput")
    bkh = nc.dram_tensor("bkh", (P, nm), f32, kind="ExternalInput")
    bvh = nc.dram_tensor("bvh", (1, CG), f32, kind="ExternalInput")
    out = nc.dram_tensor("out", (t, C), f32, kind="ExternalOutput")

    from contextlib import ExitStack

    with TileContext(nc) as tc, ExitStack() as es:
        pp = es.enter_context(tc.tile_pool(name="persist", bufs=1))
        xtpool = es.enter_context(tc.tile_pool(name="xt", bufs=16))
        epool = es.enter_context(tc.tile_pool(name="e", bufs=8))
        ytpool = es.enter_context(tc.tile_pool(name="yt", bufs=8))
        opool = es.enter_context(tc.tile_pool(name="osb", bufs=4))
        npool = es.enter_context(tc.tile_pool(name="nrm", bufs=3))
        if True:
            # ---- constants ----
            bq_sb = pp.tile([P, nm], f32, tag="bq_sb", name="bq_sb")
            nc.sync.dma_start(out=bq_sb, in_=bqh[:, :])
            bk_sb = pp.tile([P, nm], f32, tag="bk_sb", name="bk_sb")
            nc.sync.dma_start(out=bk_sb, in_=bkh[:, :])
            bv_row = pp.tile([1, CG], f32, tag="bv_row", name="bv_row")
            nc.sync.dma_start(out=bv_row, in_=bvh[:, :])
            bv_bc = pp.tile([P, CG], f32, tag="bv_bc", name="bv_bc")
            nc.gpsimd.partition_broadcast(bv_bc, bv_row)
            # tri[p, y] = 1 if y >= p else 0  (keep i_local >= j_local)
            tri = pp.tile([P, P], bf16, tag="tri", name="tri")
            nc.gpsimd.memset(tri, 1.0)
            nc.gpsimd.affine_select(
                out=tri, in_=tri, compare_op=mybir.AluOpType.is_ge,
                fill=0.0, base=0, pattern=[[1, P]], channel_multiplier=-1,
            )

            # ---- weights ----
            wq_sb, wk_sb, wv_sb = [], [], []
            for k in range(ck):
                for lst, wt, nm_ in ((wq_sb, wqT, "wq"), (wk_sb, wkT, "wk"), (wv_sb, wvT, "wv")):
                    tl = pp.tile([P, CG], bf16, tag=f"{nm_}{k}", name=f"{nm_}{k}")
                    nc.sync.dma_start(out=tl, in_=wt[k * P:(k + 1) * P, :])
                    lst.append(tl)
            wp_sb = []
            for p_ in range(nm):
                tl = pp.tile([P, C], bf16, tag=f"wp{p_}", name=f"wp{p_}")
                nc.sync.dma_start(out=tl, in_=wpT[p_ * P:(p_ + 1) * P, :])
                wp_sb.append(tl)

            # ---- stage 1: QKV ----
            qt_sb = [pp.tile([P, t], bf16, tag=f"qt{m}", name=f"qt{m}") for m in range(nm)]
            kt_sb = [pp.tile([P, t], bf16, tag=f"kt{m}", name=f"kt{m}") for m in range(nm)]
            # v tiles: (P, 8*128) bf16 — per head 64 V columns then 64 ones columns
            v_sb = [pp.tile([P, GH * 2 * D], bf16, tag=f"v{i}", name=f"v{i}") for i in range(nt)]

            with tc.tile_pool(name="qkv_ps", bufs=8, space="PSUM") as qkvpool:
                for nb in range(nib):
                    ts_ = slice(nb * NBLK, (nb + 1) * NBLK)
                    xts = []
                    for k in range(ck):
                        xt_ = xtpool.tile([P, NBLK], bf16, tag="xt", name=f"xt{nb}_{k}")
                        nc.sync.dma_start(out=xt_, in_=xT[k * P:(k + 1) * P, ts_])
                        xts.append(xt_)
                    ps_q = [qkvpool.tile([P, NBLK], f32, tag="ps", name=f"psq{nb}_{m}") for m in range(nm)]
                    ps_k = [qkvpool.tile([P, NBLK], f32, tag="ps", name=f"psk{nb}_{m}") for m in range(nm)]
                    for k in range(ck):
                        for m in range(nm):
                            nc.tensor.matmul(ps_q[m], wq_sb[k][:, m * P:(m + 1) * P], xts[k],
                                             start=(k == 0), stop=(k == ck - 1))
                        for m in range(nm):
                            nc.tensor.matmul(ps_k[m], wk_sb[k][:, m * P:(m + 1) * P], xts[k],
                                             start=(k == 0), stop=(k == ck - 1))
                    for m in range(nm):
                        nc.vector.tensor_scalar_add(qt_sb[m][:, ts_], ps_q[m], bq_sb[:, m:m + 1])
                        nc.vector.tensor_scalar_add(kt_sb[m][:, ts_], ps_k[m], bk_sb[:, m:m + 1])
                    ps_v = [qkvpool.tile([P, NBLK], f32, tag="ps", name=f"psv{nb}_{i}") for i in range(blk_t)]
                    for k in range(ck):
                        for i in range(blk_t):
                            nc.tensor.matmul(ps_v[i], xts[k][:, i * P:(i + 1) * P], wv_sb[k],
                                             start=(k == 0), stop=(k == ck - 1))
                    for i in range(blk_t):
                        vt = v_sb[nb * blk_t + i]
                        v3 = vt.rearrange("p (g d) -> p g d", d=2 * D)
                        nc.vector.tensor_add(
                            v3[:, :, 0:D],
                            ps_v[i].rearrange("p (h d) -> p h d", d=D),
                            bv_bc.rearrange("p (h d) -> p h d", d=D),
                        )
                        nc.vector.memset(v3[:, :, D:2 * D], 1.0)

            # ---- stage 2+3: attention + projection per i-block ----
            stpool = es.enter_context(tc.tile_pool(name="st_ps", bufs=3, space="PSUM"))
            avpool = es.enter_context(tc.tile_pool(name="av_ps", bufs=2, space="PSUM"))
            pjpool = es.enter_context(tc.tile_pool(name="pj_ps", bufs=2, space="PSUM"))
            for ib in range(nib):
                jt_max = blk_t * (ib + 1)
                is_ = slice(ib * NBLK, (ib + 1) * NBLK)
                yt_tiles = []
                for h in range(GH):
                    m, r = h // 2, (h % 2) * D
                    ytps = avpool.tile([P, NBLK], f32, tag="ytps", name=f"ytps{ib}_{h}")
                    e_tiles = []

                    def _vaug(jt, h=h):
                        # lhsT: head h's contiguous [V | ones] 128 columns
                        return v_sb[jt][:, h * 2 * D:(h + 1) * 2 * D]

                    for jt in range(jt_max):
                        st = stpool.tile([P, NBLK], f32, tag="st", name=f"st{ib}_{h}_{jt}")
                        nc.tensor.matmul(
                            st,
                            kt_sb[m][r:r + D, jt * P:(jt + 1) * P],
                            qt_sb[m][r:r + D, is_],
                            start=True, stop=True,
                        )
                        e = epool.tile([P, NBLK], bf16, tag="e", name=f"e{ib}_{h}_{jt}")
                        o = jt * P - ib * NBLK
                        if o < 0:
                            nc.scalar.activation(e, st, mybir.ActivationFunctionType.Exp,
                                                 scale=0.125)
                        else:
                            if o > 0:
                                nc.vector.memset(e[:, 0:o], 0.0)
                            nc.scalar.activation(e[:, o:NBLK], st[:, o:NBLK],
                                                 mybir.ActivationFunctionType.Exp, scale=0.125)
                            nc.vector.tensor_mul(e[:, o:o + P], e[:, o:o + P], tri)
                        e_tiles.append(e)
                        if jt > 0:
                            nc.tensor.matmul(ytps, _vaug(jt - 1), e_tiles[jt - 1],
                                             start=(jt - 1 == 0), stop=False)
                    nc.tensor.matmul(ytps, _vaug(jt_max - 1), e_tiles[jt_max - 1],
                                     start=(jt_max - 1 == 0), stop=True)
                    # normalize: rows D..2D of ytps all hold the softmax denominator
                    recip = npool.tile([D, NBLK], f32, tag="recip", name=f"rc{ib}_{h}")
                    nc.vector.reciprocal(recip, ytps[D:2 * D, :])
                    if h % 2 == 0:
                        yt_cur = ytpool.tile([P, NBLK], bf16, tag="yt", name=f"yt{ib}_{h // 2}")
                        yt_tiles.append(yt_cur)
                    nc.vector.tensor_mul(yt_tiles[h // 2][r:r + D, :], ytps[0:D, :], recip)
                # projection for this i-block's 4 t-tiles
                for i in range(blk_t):
                    tt = ib * blk_t + i
                    for cb in range(C // NBLK):
                        pj = pjpool.tile([P, NBLK], f32, tag="pj", name=f"pj{tt}_{cb}")
                        for p_ in range(nm):
                            nc.tensor.matmul(pj, yt_tiles[p_][:, i * P:(i + 1) * P],
                                             wp_sb[p_][:, cb * NBLK:(cb + 1) * NBLK],
                                             start=(p_ == 0), stop=(p_ == nm - 1))
                        ot = opool.tile([P, NBLK], f32, tag="osb", name=f"ot{tt}_{cb}")
                        nc.vector.tensor_copy(out=ot, in_=pj)
                        nc.sync.dma_start(out=out[tt * P:(tt + 1) * P, cb * NBLK:(cb + 1) * NBLK],
                                          in_=ot)

    nc.compile()
    return nc


def _get_nc(t=T):
    if t not in _CACHE:
        _CACHE[t] = _build_nc(t)
    return _CACHE[t]


def kernel(x, Wq, bq, Wk, bk, Wv, bv, Wp, bp):
    import ml_dtypes
    from concourse import bass_utils

    x = np.asarray(x, dtype=np.float32)
    Wq = np.asarray(Wq, dtype=np.float32)
    Wk = np.asarray(Wk, dtype=np.float32)
    Wv = np.asarray(Wv, dtype=np.float32)
    Wp = np.asarray(Wp, dtype=np.float32)
    bq = np.asarray(bq, dtype=np.float32)
    bk = np.asarray(bk, dtype=np.float32)
    bv = np.asarray(bv, dtype=np.float32)
    bp = np.asarray(bp, dtype=np.float32)

    nc = _get_nc()
    bf = ml_dtypes.bfloat16

    in_maps = []
    for core in range(N_CORES):
        b, g = core // 2, core % 2
        gs = slice(g * CG, (g + 1) * CG)
        in_maps.append({
            "xT": x[b].T.astype(bf),
            "wqT": Wq[gs, :].T.astype(bf),
            "wkT": Wk[gs, :].T.astype(bf),
            "wvT": Wv[gs, :].T.astype(bf),
            "wpT": Wp[:, gs].T.astype(bf),
            "bqh": np.ascontiguousarray(bq[gs].reshape(CG // P, P).T),
            "bkh": np.ascontiguousarray(bk[gs].reshape(CG // P, P).T),
            "bvh": bv[gs].reshape(1, CG),
        })

    res = bass_utils.run_bass_kernel_spmd(nc, in_maps, core_ids=list(range(N_CORES)),
                                          **RUN_KWARGS)
    LAST_RESULT[0] = res
    y = np.empty((B, T, C), dtype=np.float32)
    for b in range(B):
        y[b] = res.results[2 * b]["out"] + res.results[2 * b + 1]["out"] + bp
    return y


# revision 11
# speedup vs baseline: 1.6561x; 1.4518x over previous
"""Causal self-attention (B=4, T=2048, C=1024, H=16) on 8 Trainium2 NeuronCores.

Sharding: core = (b, g) with b = core//2 (batch), g = core%2 (head group of 8
heads / 512 features).  Each core computes its batch's attention for its 8
heads plus the partial output projection for its feature slice; the host sums
the two partials per batch and adds the projection bias.

Per-core kernel (all shapes hardcoded):
  inputs  xT (1024, 2048) = x[b].T          bf16
          wqT/wkT/wvT (1024, 512) = W[g].T  bf16
          wpT (512, 1024) = Wp[:, g].T      bf16
          bqh/bkh (128, 4), bvh (1, 512)    fp32
  output  out (2048, 1024) partial          fp32

Stages: QT/KT = W.T@xT (transposed layout, per-partition bias), V natural
layout followed by a 64-wide ones block; per head ST = K Q^T (K=64
contraction), exp on ScalarE with 1/sqrt(D) folded into the activation
scale, causal handled by skipping blocks above the diagonal + a triangular
mask on diagonal blocks; AV as YT' = [V_h | ones64].T @ E accumulated in
PSUM so rows 64:128 all hold the softmax denominator (reciprocal runs on 64
lanes); projection from the YT layout, host adds the two partials + bias.
"""

import sys

if "/opt/trn_rl_repo" not in sys.path:
    sys.path.insert(0, "/opt/trn_rl_repo")

import numpy as np

B, T, C, H = 4, 2048, 1024, 16
D = C // H          # 64 head dim
GH = H // 2         # 8 heads per core
CG = C // 2         # 512 features per head group
P = 128             # partitions
NBLK = 512          # free-dim block (t-block / i-block)
N_CORES = 8

_CACHE = {}
RUN_KWARGS = {}     # test harness can set {"trace": True, ...}
LAST_RESULT = [None]


def _build_nc(t=T):
    import concourse.mybir as mybir
    from concourse import bacc
    from concourse.tile import TileContext

    f32 = mybir.dt.float32
    bf16 = mybir.dt.bfloat16

    nt = t // P            # t-tiles
    nib = t // NBLK        # i-blocks
    ck = C // P            # 8 contraction tiles over C
    nm = CG // P           # 4 c'-tiles per group
    blk_t = NBLK // P      # 4 t-tiles per block

    nc = bacc.Bacc("TRN2", target_bir_lowering=False, num_devices=N_CORES)

    xT = nc.dram_tensor("xT", (C, t), bf16, kind="ExternalInput")
    wqT = nc.dram_tensor("wqT", (C, CG), bf16, kind="ExternalInput")
    wkT = nc.dram_tensor("wkT", (C, CG), bf16, kind="ExternalInput")
    wvT = nc.dram_tensor("wvT", (C, CG), bf16, kind="ExternalInput")
    wpT = nc.dram_tensor("wpT", (CG, C), bf16, kind="ExternalInput")
    bqh = nc.dram_tensor("bqh", (P, nm), f32, kind="ExternalInput")
    bkh = nc.dram_tensor("bkh", (P, nm), f32, kind="ExternalInput")
    bvh = nc.dram_tensor("bvh", (1, CG), f32, kind="ExternalInput")
    out = nc.dram_tensor("out", (t, C), f32, kind="ExternalOutput")

    from contextlib import ExitStack

    with TileContext(nc) as tc, ExitStack() as es:
        pp = es.enter_context(tc.tile_pool(name="persist", bufs=1))
        xtpool = es.enter_context(tc.tile_pool(name="xt", bufs=16))
        epool = es.enter_context(tc.tile_pool(name="e", bufs=8))
        ytpool = es.enter_context(tc.tile_pool(name="yt", bufs=8))
        opool = es.enter_context(tc.tile_pool(name="osb", bufs=4))
        npool = es.enter_context(tc.tile_pool(name="nrm", bufs=3))
        if True:
            # ---- constants ----
            bq_sb = pp.tile([P, nm], f32, tag="bq_sb", name="bq_sb")
            nc.sync.dma_start(out=bq_sb, in_=bqh[:, :])
            bk_sb = pp.tile([P, nm], f32, tag="bk_sb", name="bk_sb")
            nc.sync.dma_start(out=bk_sb, in_=bkh[:, :])
            bv_row = pp.tile([1, CG], f32, tag="bv_row", name="bv_row")
            nc.sync.dma_start(out=bv_row, in_=bvh[:, :])
            bv_bc = pp.tile([P, CG], f32, tag="bv_bc", name="bv_bc")
            nc.gpsimd.partition_broadcast(bv_bc, bv_row)
            # tri[p, y] = 1 if y >= p else 0  (keep i_local >= j_local)
            tri = pp.tile([P, P], bf16, tag="tri", name="tri")
            nc.gpsimd.memset(tri, 1.0)
            nc.gpsimd.affine_select(
                out=tri, in_=tri, compare_op=mybir.AluOpType.is_ge,
                fill=0.0, base=0, pattern=[[1, P]], channel_multiplier=-1,
            )

            # ---- weights ----
            wq_sb, wk_sb, wv_sb = [], [], []
            for k in range(ck):
                for lst, wt, nm_ in ((wq_sb, wqT, "wq"), (wk_sb, wkT, "wk"), (wv_sb, wvT, "wv")):
                    tl = pp.tile([P, CG], bf16, tag=f"{nm_}{k}", name=f"{nm_}{k}")
                    nc.sync.dma_start(out=tl, in_=wt[k * P:(k + 1) * P, :])
                    lst.append(tl)
            # ---- stage 1: QKV ----
            qt_sb = [pp.tile([P, t], bf16, tag=f"qt{m}", name=f"qt{m}") for m in range(nm)]
            kt_sb = [pp.tile([P, t], bf16, tag=f"kt{m}", name=f"kt{m}") for m in range(nm)]
            # v tiles: (P, 8*128) bf16 — per head 64 V columns then 64 ones columns
            v_sb = [pp.tile([P, GH * 2 * D], bf16, tag=f"v{i}", name=f"v{i}") for i in range(nt)]

            with tc.tile_pool(name="qkv_ps", bufs=8, space="PSUM") as qkvpool:
                for nb in range(nib):
                    ts_ = slice(nb * NBLK, (nb + 1) * NBLK)
                    xts = []
                    for k in range(ck):
                        xt_ = xtpool.tile([P, NBLK], bf16, tag="xt", name=f"xt{nb}_{k}")
                        nc.sync.dma_start(out=xt_, in_=xT[k * P:(k + 1) * P, ts_])
                        xts.append(xt_)
                    ps_q = [qkvpool.tile([P, NBLK], f32, tag="ps", name=f"psq{nb}_{m}") for m in range(nm)]
                    ps_k = [qkvpool.tile([P, NBLK], f32, tag="ps", name=f"psk{nb}_{m}") for m in range(nm)]
                    for k in range(ck):
                        for m in range(nm):
                            nc.tensor.matmul(ps_q[m], wq_sb[k][:, m * P:(m + 1) * P], xts[k],
                                             start=(k == 0), stop=(k == ck - 1))
                        for m in range(nm):
                            nc.tensor.matmul(ps_k[m], wk_sb[k][:, m * P:(m + 1) * P], xts[k],
                                             start=(k == 0), stop=(k == ck - 1))
                    for m in range(nm):
                        nc.vector.tensor_scalar_add(qt_sb[m][:, ts_], ps_q[m], bq_sb[:, m:m + 1])
                        nc.vector.tensor_scalar_add(kt_sb[m][:, ts_], ps_k[m], bk_sb[:, m:m + 1])
                    ps_v = [qkvpool.tile([P, NBLK], f32, tag="ps", name=f"psv{nb}_{i}") for i in range(blk_t)]
                    for k in range(ck):
                        for i in range(blk_t):
                            nc.tensor.matmul(ps_v[i], xts[k][:, i * P:(i + 1) * P], wv_sb[k],
                                             start=(k == 0), stop=(k == ck - 1))
                    for i in range(blk_t):
                        vt = v_sb[nb * blk_t + i]
                        v3 = vt.rearrange("p (g d) -> p g d", d=2 * D)
                        nc.vector.tensor_add(
                            v3[:, :, 0:D],
                            ps_v[i].rearrange("p (h d) -> p h d", d=D),
                            bv_bc.rearrange("p (h d) -> p h d", d=D),
                        )
                        nc.vector.memset(v3[:, :, D:2 * D], 1.0)

            wp_sb = []
            for p_ in range(nm):
                tl = pp.tile([P, C], bf16, tag=f"wp{p_}", name=f"wp{p_}")
                nc.sync.dma_start(out=tl, in_=wpT[p_ * P:(p_ + 1) * P, :])
                wp_sb.append(tl)

            # ---- stage 2+3: attention + projection per i-block ----
            stpool = es.enter_context(tc.tile_pool(name="st_ps", bufs=2, space="PSUM"))
            avpool = es.enter_context(tc.tile_pool(name="av_ps", bufs=2, space="PSUM"))
            pjpool = es.enter_context(tc.tile_pool(name="pj_ps", bufs=2, space="PSUM"))

            def emit_proj(ib, yts):
                for i in range(blk_t):
                    tt = ib * blk_t + i
                    for cb in range(C // NBLK):
                        pj = pjpool.tile([P, NBLK], f32, tag="pj", name=f"pj{tt}_{cb}")
                        for p_ in range(nm):
                            nc.tensor.matmul(pj, yts[p_][:, i * P:(i + 1) * P],
                                             wp_sb[p_][:, cb * NBLK:(cb + 1) * NBLK],
                                             start=(p_ == 0), stop=(p_ == nm - 1))
                        ot = opool.tile([P, NBLK], f32, tag="osb", name=f"ot{tt}_{cb}")
                        nc.vector.tensor_copy(out=ot, in_=pj)
                        nc.sync.dma_start(out=out[tt * P:(tt + 1) * P, cb * NBLK:(cb + 1) * NBLK],
                                          in_=ot)

            yt_hist = {}
            for ib in range(nib):
                jt_max = blk_t * (ib + 1)
                n_pair = jt_max // 2
                is_ = slice(ib * NBLK, (ib + 1) * NBLK)
                yt_tiles = []
                for h in range(GH):
                    m, r = h // 2, (h % 2) * D
                    ytps = avpool.tile([P, NBLK], f32, tag="ytps", name=f"ytps{ib}_{h}")
                    e_pairs = []

                    def _vaug(jt, h=h):
                        # lhsT: head h's contiguous [V | ones] 128 columns
                        return v_sb[jt][:, h * 2 * D:(h + 1) * 2 * D]

                    def _av(u, h=h, ib=ib, jt_max=jt_max, e_pairs=e_pairs):
                        for half in range(2):
                            jt = 2 * u + half
                            nc.tensor.matmul(ytps, _vaug(jt), e_pairs[u][:, half * NBLK:(half + 1) * NBLK],
                                             start=(jt == 0), stop=(jt == jt_max - 1))

                    for u in range(n_pair):
                        st2 = stpool.tile([P, 2 * NBLK], f32, tag="st", name=f"st{ib}_{h}_{u}")
                        for half in range(2):
                            jt = 2 * u + half
                            nc.tensor.matmul(
                                st2[:, half * NBLK:(half + 1) * NBLK],
                                kt_sb[m][r:r + D, jt * P:(jt + 1) * P],
                                qt_sb[m][r:r + D, is_],
                                start=True, stop=True,
                            )
                        e2 = epool.tile([P, 2 * NBLK], bf16, tag="e", name=f"e{ib}_{h}_{u}")
                        if (2 * u + 1) * P - ib * NBLK < 0:
                            nc.scalar.activation(e2, st2, mybir.ActivationFunctionType.Exp,
                                                 scale=0.125)
                        else:
                            for half in range(2):
                                jt = 2 * u + half
                                base = half * NBLK
                                o = jt * P - ib * NBLK
                                if o < 0:
                                    nc.scalar.activation(e2[:, base:base + NBLK],
                                                         st2[:, base:base + NBLK],
                                                         mybir.ActivationFunctionType.Exp,
                                                         scale=0.125)
                                else:
                                    if o > 0:
                                        nc.vector.memset(e2[:, base:base + o], 0.0)
                                    nc.scalar.activation(e2[:, base + o:base + NBLK],
                                                         st2[:, base + o:base + NBLK],
                                                         mybir.ActivationFunctionType.Exp,
                                                         scale=0.125)
                                    nc.vector.tensor_mul(e2[:, base + o:base + o + P],
                                                         e2[:, base + o:base + o + P], tri)
                        e_pairs.append(e2)
                        if u > 0:
                            _av(u - 1)
                    _av(n_pair - 1)
                    # normalize: rows D..2D of ytps all hold the softmax denominator
                    zsb = npool.tile([D, NBLK], f32, tag="zsb", name=f"z{ib}_{h}")
                    nc.vector.tensor_copy(out=zsb, in_=ytps[D:2 * D, :])
                    recip = npool.tile([D, NBLK], f32, tag="recip", name=f"rc{ib}_{h}")
                    nc.vector.reciprocal_approx_fast(out=recip, in_=zsb)
                    if h % 2 == 0:
                        yt_cur = ytpool.tile([P, NBLK], bf16, tag="yt", name=f"yt{ib}_{h // 2}")
                        yt_tiles.append(yt_cur)
                    nc.vector.tensor_mul(yt_tiles[h // 2][r:r + D, :], ytps[0:D, :], recip)
                yt_hist[ib] = yt_tiles
                if ib > 0:
                    emit_proj(ib - 1, yt_hist[ib - 1])
            emit_proj(nib - 1, yt_hist[nib - 1])

    nc.compile()
    return nc


def _get_nc(t=T):
    if t not in _CACHE:
        _CACHE[t] = _build_nc(t)
    return _CACHE[t]


def kernel(x, Wq, bq, Wk, bk, Wv, bv, Wp, bp):
    import ml_dtypes
    from concourse import bass_utils

    x = np.asarray(x, dtype=np.float32)
    Wq = np.asarray(Wq, dtype=np.float32)
    Wk = np.asarray(Wk, dtype=np.float32)
    Wv = np.asarray(Wv, dtype=np.float32)
    Wp = np.asarray(Wp, dtype=np.float32)
    bq = np.asarray(bq, dtype=np.float32)
    bk = np.asarray(bk, dtype=np.float32)
    bv = np.asarray(bv, dtype=np.float32)
    bp = np.asarray(bp, dtype=np.float32)

    nc = _get_nc()
    bf = ml_dtypes.bfloat16

    in_maps = []
    for core in range(N_CORES):
        b, g = core // 2, core % 2
        gs = slice(g * CG, (g + 1) * CG)
        in_maps.append({
            "xT": x[b].T.astype(bf),
            "wqT": Wq[gs, :].T.astype(bf),
            "wkT": Wk[gs, :].T.astype(bf),
            "wvT": Wv[gs, :].T.astype(bf),
            "wpT": Wp[:, gs].T.astype(bf),
            "bqh": np.ascontiguousarray(bq[gs].reshape(CG // P, P).T),
            "bkh": np.ascontiguousarray(bk[gs].reshape(CG // P, P).T),
            "bvh": bv[gs].reshape(1, CG),
        })

    res = bass_utils.run_bass_kernel_spmd(nc, in_maps, core_ids=list(range(N_CORES)),
                                          **RUN_KWARGS)
    LAST_RESULT[0] = res
    y = np.empty((B, T, C), dtype=np.float32)
    for b in range(B):
        y[b] = res.results[2 * b]["out"] + res.results[2 * b + 1]["out"] + bp
    return y
